# revision 1
# baseline (speedup 1.0000x reference)
"""Trainium2 Bass kernel for nn_CrossAttentionModule (B=4, C=2048, H=W=32).

The module is two independent cross-attention streams per batch element
(RGB queries over index features, and index queries over RGB features).
That yields 8 perfectly independent units = 4 batches x 2 streams; one
unit per NeuronCore, zero collectives.

Per-core program (all matmuls bf16, fp32 PSUM accumulate):
  Q  = (Wq/sqrt(C)) @ Xq + bq/sqrt(C)        [C, N]   (scale folded on host)
  K  = Wk @ Xkv + bk                          [C, N]
  VT = Xkv^T @ Wv^T + 1 x bv                  [N, C]   (computed directly
       transposed; bias added as a K=1 rank-1 matmul into the same PSUM)
  S  = Q^T K                                  [N, N]
  A  = softmax(S, axis=-1)  (row max via negate-reduce, Exp activation with
       fused accum row-sum, reciprocal, row scale)
  AT = A^T via PE transposes (128x128 tiles)
  O  = (VT)^T @ AT = V @ A^T                  [C, N]  fp32 out

Host side: pre-transposes/pre-tiles the weights into the exact slab layout
the kernel streams (every DMA is contiguous), casts to bf16, distributes
the 8 units across cores, and reassembles the 4 reference outputs.
"""

import math
from functools import lru_cache

import ml_dtypes
import numpy as np

B, C, HW, N = 4, 2048, 32, 1024
P = 128
CT = C // P           # 16 channel tiles
NT = N // P           # 8 pixel tiles
KHALF = 512           # moving free dim per matmul
CG = C // KHALF       # 4 output-channel groups for the VT conv

_BF16 = ml_dtypes.bfloat16


def _build_program():
    import concourse.mybir as mybir
    import concourse.tile as tile
    from concourse import bacc
    from concourse.masks import make_identity

    dtb = mybir.dt.bfloat16
    dtf = mybir.dt.float32
    PS = mybir.MemorySpace.PSUM if hasattr(mybir, "MemorySpace") else None

    import concourse.bass as bass

    nc = bacc.Bacc("TRN2", target_bir_lowering=False, debug=False)

    xq_d = nc.declare_dram_parameter("xq", [C, N], dtb, isOutput=False)
    xkv_d = nc.declare_dram_parameter("xkv", [C, N], dtb, isOutput=False)
    wq_d = nc.declare_dram_parameter("wq", [CT, P, CT, P], dtb, isOutput=False)
    wk_d = nc.declare_dram_parameter("wk", [CT, P, CT, P], dtb, isOutput=False)
    wv_d = nc.declare_dram_parameter("wv", [CG, P, CT, KHALF], dtb, isOutput=False)
    bq_d = nc.declare_dram_parameter("bq", [P, CT], dtf, isOutput=False)
    bk_d = nc.declare_dram_parameter("bk", [P, CT], dtf, isOutput=False)
    bv_d = nc.declare_dram_parameter("bv", [1, C], dtb, isOutput=False)
    out_d = nc.declare_dram_parameter("out", [C, N], dtf, isOutput=True)

    with tile.TileContext(nc) as tc:
        with (
            tc.tile_pool(name="const", bufs=1) as const_pool,
            tc.tile_pool(name="big", bufs=1) as big_pool,
            tc.tile_pool(name="wqk", bufs=3) as wqk_pool,
            tc.tile_pool(name="stat", bufs=8) as stat_pool,
            tc.tile_pool(name="ostage", bufs=3) as ostage_pool,
            tc.tile_pool(name="psmm", bufs=2, space=bass.MemorySpace.PSUM) as psmm,
            tc.tile_pool(name="pss", bufs=2, space=bass.MemorySpace.PSUM) as pss,
            tc.tile_pool(name="pstr", bufs=2, space=bass.MemorySpace.PSUM) as pstr,
        ):
            # constants
            identity = const_pool.tile([P, P], dtb)
            make_identity(nc, identity[:])
            ones_row = const_pool.tile([1, P], dtb)
            nc.gpsimd.memset(ones_row[:], 1.0)
            bq_sb = const_pool.tile([P, CT], dtf)
            nc.sync.dma_start(bq_sb[:], bq_d[:])
            bk_sb = const_pool.tile([P, CT], dtf)
            nc.sync.dma_start(bk_sb[:], bk_d[:])
            bv_sb = const_pool.tile([1, C], dtb)
            nc.sync.dma_start(bv_sb[:], bv_d[:])

            # persistent activations
            q_sb = big_pool.tile([P, CT, N], dtb)     # Q[c, n]
            k_sb = big_pool.tile([P, CT, N], dtb)     # K[c, n]
            vt_sb = big_pool.tile([P, NT, C], dtb)    # V^T[m, c]

            def conv_qk(x_sb, w_dram, b_sb, dst):
                # dst[c, n] = W @ x + b, computed as 16 output-channel tiles
                for ot in range(CT):
                    w_slab = wqk_pool.tile([P, CT, P], dtb, tag="wslab")
                    nc.sync.dma_start(w_slab[:], w_dram[ot])
                    ps0 = psmm.tile([P, KHALF], dtf, tag="mm")
                    ps1 = psmm.tile([P, KHALF], dtf, tag="mm")
                    for kt in range(CT):
                        nc.tensor.matmul(
                            ps0[:], w_slab[:, kt, :], x_sb[:, kt, 0:KHALF],
                            start=(kt == 0), stop=(kt == CT - 1),
                        )
                        nc.tensor.matmul(
                            ps1[:], w_slab[:, kt, :], x_sb[:, kt, KHALF:N],
                            start=(kt == 0), stop=(kt == CT - 1),
                        )
                    nc.vector.tensor_scalar_add(
                        dst[:, ot, 0:KHALF], ps0[:], b_sb[:, ot : ot + 1]
                    )
                    nc.vector.tensor_scalar_add(
                        dst[:, ot, KHALF:N], ps1[:], b_sb[:, ot : ot + 1]
                    )

            # ---- phase 1: Q conv (xq only lives here) ----
            with tc.tile_pool(name="xq", bufs=1) as xq_pool:
                xq_sb = xq_pool.tile([P, CT, N], dtb)
                for kt in range(CT):
                    nc.sync.dma_start(
                        xq_sb[:, kt, :], xq_d[kt * P : (kt + 1) * P, :]
                    )
                conv_qk(xq_sb, wq_d, bq_sb, q_sb)

            # ---- phases 2+3: K conv and VT conv (xkv lives here) ----
            with tc.tile_pool(name="xkv", bufs=1) as xkv_pool:
                xkv_sb = xkv_pool.tile([P, CT, N], dtb)
                for kt in range(CT):
                    nc.sync.dma_start(
                        xkv_sb[:, kt, :], xkv_d[kt * P : (kt + 1) * P, :]
                    )
                conv_qk(xkv_sb, wk_d, bk_sb, k_sb)

                # VT conv: VT[m, c] = sum_ci Xkv[ci, m] WvT[ci, c] + bv[c]
                with tc.tile_pool(name="wv", bufs=8) as wv_pool:
                    for cg in range(CG):
                        # four quarter-slabs of 4 kt each, all resident for
                        # the whole cg (each is reused by all 8 m-tiles)
                        quarters = []
                        for qh in range(4):
                            sl = wv_pool.tile([P, 4, KHALF], dtb, tag="wv")
                            nc.sync.dma_start(
                                sl[:], wv_d[cg][:, qh * 4 : (qh + 1) * 4, :]
                            )
                            quarters.append(sl)
                        for mt in range(NT):
                            ps = psmm.tile([P, KHALF], dtf, tag="mm")
                            for kt in range(CT):
                                nc.tensor.matmul(
                                    ps[:],
                                    xkv_sb[:, kt, mt * P : (mt + 1) * P],
                                    quarters[kt // 4][:, kt % 4, :],
                                    start=(kt == 0), stop=False,
                                )
                            # bias as rank-1 update: ones^T x bv_slice
                            nc.tensor.matmul(
                                ps[:],
                                ones_row[:],
                                bv_sb[:, cg * KHALF : (cg + 1) * KHALF],
                                start=False, stop=True,
                            )
                            nc.vector.tensor_copy(
                                vt_sb[:, mt, cg * KHALF : (cg + 1) * KHALF], ps[:]
                            )

            # ---- phases 4-6: S = Q^T K, softmax, A^T, O = V A^T ----
            with tc.tile_pool(name="attn", bufs=1) as attn_pool:
                a_sb = attn_pool.tile([P, NT, N], dtb)   # A[nq, nk]
                at_sb = attn_pool.tile([P, NT, N], dtb)  # A^T[m, nq]

                for qt in range(NT):
                    ps = pss.tile([P, N], dtf, tag="s")  # two banks
                    for nh in range(2):
                        for kt in range(CT):
                            nc.tensor.matmul(
                                ps[:, nh * KHALF : (nh + 1) * KHALF],
                                q_sb[:, kt, qt * P : (qt + 1) * P],
                                k_sb[:, kt, nh * KHALF : (nh + 1) * KHALF],
                                start=(kt == 0), stop=(kt == CT - 1),
                            )
                    nmax = stat_pool.tile([P, 1], dtf, tag="nmax")
                    nc.vector.reduce_max(
                        nmax[:], ps[:], axis=mybir.AxisListType.X, negate=True
                    )
                    rsum = stat_pool.tile([P, 1], dtf, tag="rsum")
                    nc.scalar.activation(
                        a_sb[:, qt, :], ps[:],
                        mybir.ActivationFunctionType.Exp,
                        bias=nmax[:], scale=1.0, accum_out=rsum[:],
                    )
                    rinv = stat_pool.tile([P, 1], dtf, tag="rinv")
                    nc.vector.reciprocal(rinv[:], rsum[:])
                    nc.vector.tensor_scalar_mul(
                        a_sb[:, qt, :], a_sb[:, qt, :], rinv[:]
                    )
                    # transpose this row-block of A into AT's column block
                    for mt in range(NT):
                        tp = pstr.tile([P, P], dtb, tag="tr")
                        nc.tensor.transpose(
                            tp[:], a_sb[:, qt, mt * P : (mt + 1) * P], identity[:]
                        )
                        nc.vector.tensor_copy(
                            at_sb[:, mt, qt * P : (qt + 1) * P], tp[:]
                        )

                # O = V @ A^T
                for ct in range(CT):
                    o_stage = ostage_pool.tile([P, N], dtf, tag="o")
                    for nh in range(2):
                        ps = psmm.tile([P, KHALF], dtf, tag="mm")
                        for mt in range(NT):
                            nc.tensor.matmul(
                                ps[:],
                                vt_sb[:, mt, ct * P : (ct + 1) * P],
                                at_sb[:, mt, nh * KHALF : (nh + 1) * KHALF],
                                start=(mt == 0), stop=(mt == NT - 1),
                            )
                        nc.vector.tensor_copy(
                            o_stage[:, nh * KHALF : (nh + 1) * KHALF], ps[:]
                        )
                    nc.sync.dma_start(out_d[ct * P : (ct + 1) * P, :], o_stage[:])

    nc.compile()
    return nc


@lru_cache(maxsize=1)
def _get_nc():
    return _build_program()


def _prep_wqk(W, b, scale):
    WT = np.ascontiguousarray(W.T) * scale  # [c_in, c_out]
    wt = np.ascontiguousarray(
        WT.reshape(CT, P, CT, P).transpose(2, 1, 0, 3)
    ).astype(_BF16)  # [ot, ci, kt, o]
    bp = np.ascontiguousarray((b * scale).reshape(CT, P).T).astype(np.float32)
    return wt, bp


def _prep_wv(W, b):
    WT = np.ascontiguousarray(W.T)  # [c_in, c_out]
    wt = np.ascontiguousarray(
        WT.reshape(CT, P, CG, KHALF).transpose(2, 1, 0, 3)
    ).astype(_BF16)  # [cg, ci, kt, co]
    bv = np.ascontiguousarray(b.reshape(1, C)).astype(_BF16)
    return wt, bv


def _run(inputs, trace=False):
    from concourse.bass_utils import run_bass_kernel_spmd

    F_rgb = np.asarray(inputs["F_rgb"], dtype=np.float32)
    F_ind = np.asarray(inputs["F_indices"], dtype=np.float32)

    scale = 1.0 / math.sqrt(C)
    # stream 0: rgb queries attend over index features
    wq0, bq0 = _prep_wqk(np.asarray(inputs["W_q_rgb"], np.float32),
                         np.asarray(inputs["b_q_rgb"], np.float32), scale)
    wk0, bk0 = _prep_wqk(np.asarray(inputs["W_k_ind"], np.float32),
                         np.asarray(inputs["b_k_ind"], np.float32), 1.0)
    wv0, bv0 = _prep_wv(np.asarray(inputs["W_v_ind"], np.float32),
                        np.asarray(inputs["b_v_ind"], np.float32))
    # stream 1: index queries attend over rgb features
    wq1, bq1 = _prep_wqk(np.asarray(inputs["W_q_ind"], np.float32),
                         np.asarray(inputs["b_q_ind"], np.float32), scale)
    wk1, bk1 = _prep_wqk(np.asarray(inputs["W_k_rgb"], np.float32),
                         np.asarray(inputs["b_k_rgb"], np.float32), 1.0)
    wv1, bv1 = _prep_wv(np.asarray(inputs["W_v_rgb"], np.float32),
                        np.asarray(inputs["b_v_rgb"], np.float32))

    rgb_flat = [np.ascontiguousarray(F_rgb[b].reshape(C, N)).astype(_BF16)
                for b in range(B)]
    ind_flat = [np.ascontiguousarray(F_ind[b].reshape(C, N)).astype(_BF16)
                for b in range(B)]

    in_maps = []
    for b in range(B):  # cores 0-3: stream 0
        in_maps.append(dict(xq=rgb_flat[b], xkv=ind_flat[b], wq=wq0, wk=wk0,
                            wv=wv0, bq=bq0, bk=bk0, bv=bv0))
    for b in range(B):  # cores 4-7: stream 1
        in_maps.append(dict(xq=ind_flat[b], xkv=rgb_flat[b], wq=wq1, wk=wk1,
                            wv=wv1, bq=bq1, bk=bk1, bv=bv1))

    nc = _get_nc()
    res = run_bass_kernel_spmd(nc, in_maps, core_ids=list(range(8)), trace=trace)

    O1 = np.stack([res.results[b]["out"].reshape(C, HW, HW) for b in range(B)])
    O2 = np.stack([res.results[4 + b]["out"].reshape(C, HW, HW) for b in range(B)])
    F_final = O1 + O2
    attention_weights = np.stack([O1, O2], axis=1)
    return (F_final, F_rgb, F_ind, attention_weights), res


def kernel(**inputs):
    outs, _ = _run(inputs, trace=False)
    return outs


def kernel_profiled(**inputs):
    outs, res = _run(inputs, trace=True)
    return outs, res
